# revision 17
# baseline (speedup 1.0000x reference)
"""GAT-style message passing (nn_MicroConv) on 8 Trainium2 NeuronCores.

Strategy (dst-node partition, per the sharding hint's second option):
  - Each core owns N_DST/8 destination nodes and all edges into them.
  - Host: sort edges by dst, bucket into 32-dst windows, assign windows to
    fixed-size slot ranks (sorted by size so every core shares one compiled
    schedule), pad edge tiles of 128 to the slot grid with sentinel edges.
  - Device phase 0: per-core replicated node transform producing a gather
    table [N_src+1, 132] = [fs (128 cols) | e_src (4 cols)] plus a local
    e_dst table [n_dst_core, 4].
  - Device main pass: batched indirect-DMA gathers of 528B rows per edge,
    CCE-add fused e_dst gather, leaky-relu+exp on compact logits, per-head
    scaling, and segment reduction on the PE array via on-chip indicator
    matrices (one matmul per 128-edge tile, accumulating in PSUM per
    32-dst window, 4 windows col-tiled per PSUM bank).
  - Epilogue: reciprocal of the denominator column block, scale + ReLU,
    DMA out in slot order; host unpermutes rows to original dst ids.
"""

import math
import numpy as np

from concourse import bacc, bass, mybir, tile
from concourse.bass import IndirectOffsetOnAxis
from concourse.bass_utils import run_bass_kernel_spmd

# ---------------------------------------------------------------- constants
N_CORES = 8
H = 4          # heads
D = 32         # feats per head
HD = H * D     # 128
TC = HD + H    # 132 table columns: [fs | e_src]
W_DST = 32     # dst nodes per window (matmul indicator width)
PGROUP = 4     # windows per PSUM tile (4*32 = 128 partitions)
CHUNK = 32     # slots (128-edge tiles) per gather instruction
NEG_SLOPE = 0.2
SENT_ESRC = -1.0e30
DT = mybir.dt.float32
NP_DT = np.float32
F32 = mybir.dt.float32
I32 = mybir.dt.int32
DEBUG_DUMPS = False


def _cdiv(a, b):
    return (a + b - 1) // b


# ---------------------------------------------------------------- host prep
def _prep(feat_src, feat_dst, w_src, w_dst, attn, src_idx, dst_idx, n_cores):
    n_src, d_in = feat_src.shape
    n_dst = feat_dst.shape[0]
    assert d_in % 128 == 0
    fch = d_in // 128

    ndc = _cdiv(n_dst, n_cores)                    # dsts per core
    ndc_pad = _cdiv(ndc, PGROUP * W_DST) * PGROUP * W_DST
    nwin = ndc_pad // W_DST
    nsrc_pad = _cdiv(n_src, 128) * 128
    sent_row = nsrc_pad                            # sentinel table row id
    nt_src = nsrc_pad // 128
    nt_dst = ndc_pad // 128

    # ---- edge sort by dst
    perm = np.argsort(dst_idx, kind="stable")
    ds = dst_idx[perm]
    ss = src_idx[perm]

    counts = np.zeros((n_cores, nwin), np.int64)
    per_core = []
    for c in range(n_cores):
        lo, hi = np.searchsorted(ds, [c * ndc, min((c + 1) * ndc, n_dst)])
        d_loc = (ds[lo:hi] - c * ndc).astype(np.int64)
        s_loc = ss[lo:hi].astype(np.int64)
        win = d_loc // W_DST
        counts[c] = np.bincount(win, minlength=nwin)
        per_core.append((d_loc, s_loc, win))

    order = np.argsort(-counts, axis=1, kind="stable")     # [n_cores, nwin]
    sorted_counts = np.take_along_axis(counts, order, axis=1)
    rank_max = sorted_counts.max(axis=0)                   # [nwin]
    t_r = np.maximum(1, _cdiv(rank_max, 128)).astype(np.int64)  # tiles/slotrank
    slot_base = np.concatenate([[0], np.cumsum(t_r)])
    stot = int(slot_base[-1])
    n_chunks = _cdiv(stot, CHUNK)
    stot_pad = n_chunks * CHUNK

    # schedule shared by all cores: slot -> (window rank, tile, ntiles)
    slot_sched = []
    for r in range(nwin):
        for t in range(int(t_r[r])):
            slot_sched.append((r, t, int(t_r[r])))
    assert len(slot_sched) == stot

    # ---- per-core edge slot arrays
    idxs_h, idxd_h, dloc_h = [], [], []
    for c in range(n_cores):
        d_loc, s_loc, win = per_core[c]
        e_src_ids = np.full((stot_pad, 128), sent_row, np.int32)
        e_dst_ids = np.zeros((stot_pad, 128), np.int32)
        e_dloc = np.zeros((stot_pad, 128), NP_DT)
        if len(d_loc):
            rank = np.empty(nwin, np.int64)
            rank[order[c]] = np.arange(nwin)
            win_start = np.concatenate([[0], np.cumsum(counts[c])[:-1]])
            posw = np.arange(len(d_loc)) - win_start[win]
            r_of = rank[win]
            slot = slot_base[r_of] + posw // 128
            lane = posw % 128
            e_src_ids[slot, lane] = s_loc
            e_dst_ids[slot, lane] = d_loc
            e_dloc[slot, lane] = (d_loc - win * W_DST).astype(NP_DT)
        # [n_chunks, 128, CHUNK]: arr[i, p, j] = slot i*CHUNK+j, lane p
        def _pack(a):
            return np.ascontiguousarray(
                a.reshape(n_chunks, CHUNK, 128).transpose(0, 2, 1)
            )
        idxs_h.append(_pack(e_src_ids))
        idxd_h.append(_pack(e_dst_ids))
        dloc_h.append(_pack(e_dloc))

    # ---- transposed feature tiles
    def _tiles(feat, npad):
        f = np.zeros((npad, d_in), np.float32)
        f[: feat.shape[0]] = feat
        # [nt, FCH, 128(f), 128(n)]
        return np.ascontiguousarray(
            f.reshape(npad // 128, 128, fch, 128).transpose(0, 2, 3, 1)
        )

    fsT = _tiles(feat_src, nsrc_pad)
    # feat_dst shard rows permuted into slot (sorted-window) order so the
    # e_dst table comes out slot-ordered with compile-time addresses
    fdT = []
    for c in range(n_cores):
        n_here = min(ndc, n_dst - c * ndc)
        fd_slot = np.zeros((ndc_pad, d_in), np.float32)
        for r in range(nwin):
            w = int(order[c][r])
            d0 = w * W_DST
            n = min(W_DST, n_here - d0)
            if n > 0:
                fd_slot[r * W_DST : r * W_DST + n] = \
                    feat_dst[c * ndc + d0 : c * ndc + d0 + n]
        fdT.append(_tiles(fd_slot, ndc_pad))

    # ---- attention selector matrices (pure relayout of attn input)
    a_src = np.zeros((HD, H), np.float32)
    a_dst = np.zeros((HD, H), np.float32)
    for h in range(H):
        a_dst[h * D : (h + 1) * D, h] = attn[h, :D]
        a_src[h * D : (h + 1) * D, h] = attn[h, D:]

    cfg = dict(
        n_src=n_src, n_dst=n_dst, d_in=d_in, fch=fch, ndc=ndc,
        ndc_pad=ndc_pad, nwin=nwin, nsrc_pad=nsrc_pad, sent_row=sent_row,
        nt_src=nt_src, nt_dst=nt_dst, stot=stot, stot_pad=stot_pad,
        n_chunks=n_chunks, slot_sched=slot_sched, n_cores=n_cores,
    )
    common = dict(
        wsrc=np.ascontiguousarray(w_src, np.float32),
        wsrcT=np.ascontiguousarray(w_src.T, np.float32),
        wdstT=np.ascontiguousarray(w_dst.T, np.float32),
        asrc=np.ascontiguousarray(a_src),
        adst=np.ascontiguousarray(a_dst),
        fsT=fsT,
    )
    in_maps = []
    for c in range(n_cores):
        m = dict(common)
        m["fdT"] = fdT[c]
        m["idxs"] = idxs_h[c]
        m["dloc"] = dloc_h[c]
        in_maps.append(m)
    return cfg, in_maps, order


# ---------------------------------------------------------------- device kernel
def _build(nc, tc, cfg):
    fch = cfg["fch"]
    d_in = cfg["d_in"]
    nwin = cfg["nwin"]

    # I/O
    fsT = nc.dram_tensor("fsT", [cfg["nt_src"], fch, 128, 128], F32,
                         kind="ExternalInput")
    fdT = nc.dram_tensor("fdT", [cfg["nt_dst"], fch, 128, 128], F32,
                         kind="ExternalInput")
    wsrc = nc.dram_tensor("wsrc", [d_in, HD], F32, kind="ExternalInput")
    wsrcT = nc.dram_tensor("wsrcT", [HD, d_in], F32, kind="ExternalInput")
    wdstT = nc.dram_tensor("wdstT", [HD, d_in], F32, kind="ExternalInput")
    asrc = nc.dram_tensor("asrc", [HD, H], F32, kind="ExternalInput")
    adst = nc.dram_tensor("adst", [HD, H], F32, kind="ExternalInput")
    idxs = nc.dram_tensor("idxs", [cfg["n_chunks"], 128, CHUNK], I32,
                          kind="ExternalInput")
    dloc = nc.dram_tensor("dloc", [cfg["n_chunks"], 128, CHUNK], DT,
                          kind="ExternalInput")
    out = nc.dram_tensor("out", [cfg["ndc_pad"], HD], F32,
                         kind="ExternalOutput")

    tabkind = "ExternalOutput" if DEBUG_DUMPS else "Internal"
    tab = nc.dram_tensor("tab", [cfg["nsrc_pad"] + 1, TC], DT, kind=tabkind)
    edt = nc.dram_tensor("edt", [cfg["ndc_pad"], H], DT, kind=tabkind)
    if DEBUG_DUMPS:
        dgt = nc.dram_tensor("dgt", [cfg["n_chunks"], 128, CHUNK * TC], DT,
                             kind="ExternalOutput")
        dsbt = nc.dram_tensor("dsbt", [cfg["n_chunks"], 128, CHUNK * W_DST],
                              DT, kind="ExternalOutput")

    import contextlib
    ctx = contextlib.ExitStack()
    with ctx:
        const = ctx.enter_context(tc.tile_pool(name="const", bufs=1))
        sb = ctx.enter_context(tc.tile_pool(name="sb", bufs=3))
        gp = ctx.enter_context(tc.tile_pool(name="gp", bufs=4))
        pp = ctx.enter_context(tc.tile_pool(name="pp", bufs=3, space="PSUM"))

        # ---------------- setup: W_ext = [w_src | M_src], M_dst
        wsT_sb = const.tile([128, d_in], F32, tag="wsT")
        wdT_sb = const.tile([128, d_in], F32, tag="wdT")
        asrc_sb = const.tile([128, H], F32, tag="asrc")
        adst_sb = const.tile([128, H], F32, tag="adst")
        nc.sync.dma_start(out=wsT_sb[:], in_=wsrcT[:, :])
        nc.sync.dma_start(out=wdT_sb[:], in_=wdstT[:, :])
        nc.sync.dma_start(out=asrc_sb[:], in_=asrc[:, :])
        nc.sync.dma_start(out=adst_sb[:], in_=adst[:, :])

        wext = []
        mdst = []
        for k in range(fch):
            we = const.tile([128, TC], F32, tag=f"wext{k}")
            nc.sync.dma_start(out=we[:, :HD],
                              in_=wsrc[k * 128 : (k + 1) * 128, :])
            pm = pp.tile([128, 512], F32, tag="acc")
            nc.tensor.matmul(pm[:, :H], wsT_sb[:, k * 128 : (k + 1) * 128],
                             asrc_sb[:], start=True, stop=True)
            nc.vector.tensor_copy(we[:, HD:TC], pm[:, :H])
            wext.append(we)

            md = const.tile([128, H], F32, tag=f"mdst{k}")
            pm2 = pp.tile([128, 512], F32, tag="acc")
            nc.tensor.matmul(pm2[:, :H], wdT_sb[:, k * 128 : (k + 1) * 128],
                             adst_sb[:], start=True, stop=True)
            nc.vector.tensor_copy(md[:], pm2[:, :H])
            mdst.append(md)

        iota_i = const.tile([128, W_DST], I32, tag="iota_i")
        iota_f = const.tile([128, W_DST], DT, tag="iota_f")
        nc.gpsimd.iota(iota_i[:], pattern=[[1, W_DST]], base=0,
                       channel_multiplier=0)
        nc.vector.tensor_copy(iota_f[:], iota_i[:])

        # ---------------- phase B: e_dst table
        for j in range(cfg["nt_dst"]):
            lh = sb.tile([128, fch * 128], F32, tag="lh")
            nc.sync.dma_start(out=lh[:].rearrange("p (k n) -> p k n", n=128),
                              in_=fdT[j].rearrange("k p n -> p k n"))
            pb = pp.tile([128, 512], F32, tag="acc")
            for k in range(fch):
                nc.tensor.matmul(pb[:, :H], lh[:, k * 128 : (k + 1) * 128],
                                 mdst[k][:], start=(k == 0),
                                 stop=(k == fch - 1))
            eb = sb.tile([128, H], DT, tag="eb")
            nc.vector.tensor_copy(eb[:], pb[:, :H])
            nc.sync.dma_start(out=edt[j * 128 : (j + 1) * 128, :], in_=eb[:])

        # ---------------- phase C: gather table [fs | e_src]
        for j in range(cfg["nt_src"]):
            lh = sb.tile([128, fch * 128], F32, tag="lh")
            nc.sync.dma_start(out=lh[:].rearrange("p (k n) -> p k n", n=128),
                              in_=fsT[j].rearrange("k p n -> p k n"))
            pc = pp.tile([128, 512], F32, tag="acc")
            for k in range(fch):
                nc.tensor.matmul(pc[:, :TC], lh[:, k * 128 : (k + 1) * 128],
                                 wext[k][:], start=(k == 0),
                                 stop=(k == fch - 1))
            tout = sb.tile([128, TC], DT, tag="tout")
            nc.vector.tensor_copy(tout[:], pc[:, :TC])
            nc.sync.dma_start(out=tab[j * 128 : (j + 1) * 128, :], in_=tout[:])

        # sentinel row: fs = 0, e_src = SENT_ESRC
        st = const.tile([1, TC], DT, tag="sent")
        nc.vector.memset(st[:, :HD], 0.0)
        nc.vector.memset(st[:, HD:TC], SENT_ESRC)
        nc.sync.dma_start(out=tab[cfg["nsrc_pad"] : cfg["nsrc_pad"] + 1, :],
                          in_=st[:])

        tc.strict_bb_all_engine_barrier()

        # ---------------- main pass
        sched = cfg["slot_sched"]
        psg = {}
        ewin = {}
        from concourse.masks import make_identity
        ident = const.tile([128, 128], F32, tag="ident")
        make_identity(nc, ident[:])
        for i in range(cfg["n_chunks"]):
            gt = gp.tile([128, CHUNK * TC], DT, tag="gt")
            ixs = sb.tile([128, CHUNK], I32, tag="ixs")
            dl = sb.tile([128, CHUNK], DT, tag="dl")
            nc.sync.dma_start(out=ixs[:], in_=idxs[i])
            nc.sync.dma_start(out=dl[:], in_=dloc[i])

            gt3 = gt[:].rearrange("p (s c) -> p s c", c=TC)
            # HW indirect DMA consumes one offset per output partition, so
            # gather one 128-edge slot per instruction.
            for sl in range(CHUNK):
                nc.gpsimd.indirect_dma_start(
                    out=gt[:, sl * TC : (sl + 1) * TC], out_offset=None,
                    in_=tab[:, :],
                    in_offset=IndirectOffsetOnAxis(ap=ixs[:, sl : sl + 1],
                                                   axis=0))


            # indicator S: [128, CHUNK * W_DST]
            sbt = sb.tile([128, CHUNK * W_DST], DT, tag="sbt")
            nc.vector.tensor_tensor(
                out=sbt[:].rearrange("p (s w) -> p s w", w=W_DST),
                in0=iota_f[:].rearrange("p (o w) -> p o w", o=1).to_broadcast(
                    [128, CHUNK, W_DST]),
                in1=dl[:].rearrange("p (s o) -> p s o", o=1).to_broadcast(
                    [128, CHUNK, W_DST]),
                op=mybir.AluOpType.is_equal)

            # e_dst broadcast onto logit columns: S^T (PE transpose) @
            # edt window slice, added to the gathered e_src columns
            for sl in range(CHUNK):
                s = i * CHUNK + sl
                if s >= cfg["stot"]:
                    break
                r, t, tr = sched[s]
                if t == 0:
                    ew = sb.tile([W_DST, H], DT, tag="ewin", name=f"ew{r}")
                    nc.sync.dma_start(
                        out=ew[:], in_=edt[r * W_DST:(r + 1) * W_DST, :])
                    ewin[r] = ew
                ptr = pp.tile([W_DST, 128], F32, tag="tr", name=f"ptr{s}",
                              bufs=2)
                nc.tensor.transpose(ptr[:], sbt[:, sl * W_DST:(sl + 1) * W_DST],
                                    ident[:])
                stx = sb.tile([W_DST, 128], DT, tag="stx")
                nc.vector.tensor_copy(stx[:], ptr[:])
                peb = pp.tile([128, H], F32, tag="ebp", name=f"peb{s}", bufs=2)
                nc.tensor.matmul(peb[:], stx[:], ewin[r][:],
                                 start=True, stop=True)
                nc.vector.tensor_tensor(out=gt3[:, sl, HD:TC],
                                        in0=gt3[:, sl, HD:TC],
                                        in1=peb[:], op=mybir.AluOpType.add)
                if t == tr - 1:
                    ewin.pop(r, None)

            # leaky relu + exp on logit columns (in place)
            ev = gt3[:, :, HD:TC]
            tmp = sb.tile([128, CHUNK * H], DT, tag="tmp")
            nc.vector.tensor_scalar(out=tmp[:], in0=ev, scalar1=NEG_SLOPE,
                                    scalar2=None, op0=mybir.AluOpType.mult)
            nc.vector.tensor_tensor(out=ev, in0=ev,
                                    in1=tmp[:].rearrange(
                                        "p (s h) -> p s h", h=H),
                                    op=mybir.AluOpType.max)
            nc.scalar.activation(ev, ev, mybir.ActivationFunctionType.Exp)

            # scale fs columns by per-head ex
            for h in range(H):
                fv = gt3[:, :, h * D : (h + 1) * D]
                xv = gt3[:, :, HD + h : HD + h + 1].to_broadcast(
                    [128, CHUNK, D])
                nc.vector.tensor_tensor(out=fv, in0=fv, in1=xv,
                                        op=mybir.AluOpType.mult)

            if DEBUG_DUMPS:
                nc.sync.dma_start(out=dgt[i], in_=gt[:])
                nc.sync.dma_start(out=dsbt[i], in_=sbt[:])

            # segment matmuls
            for sl in range(CHUNK):
                s = i * CHUNK + sl
                if s >= cfg["stot"]:
                    break
                r, t, tr = sched[s]
                g, q = r // PGROUP, r % PGROUP
                if q == 0 and t == 0:
                    psg[g] = pp.tile([128, 512], F32, tag="acc",
                                     name=f"psg{g}")
                nc.tensor.matmul(
                    psg[g][q * W_DST : (q + 1) * W_DST, :TC],
                    sbt[:, sl * W_DST : (sl + 1) * W_DST],
                    gt[:, sl * TC : (sl + 1) * TC],
                    start=(t == 0), stop=(t == tr - 1),
                    tile_position=(0, q * W_DST))
                if q == PGROUP - 1 and t == tr - 1:
                    # epilogue for group g
                    pt = psg.pop(g)
                    dmx = sb.tile([128, H], F32, tag="dmx")
                    rcp = sb.tile([128, H], F32, tag="rcp")
                    nc.vector.tensor_scalar(out=dmx[:], in0=pt[:, HD:TC],
                                            scalar1=1e-30, scalar2=None,
                                            op0=mybir.AluOpType.max)
                    nc.vector.reciprocal(rcp[:], dmx[:])
                    ot = sb.tile([128, HD], F32, tag="ot")
                    for h in range(H):
                        nc.vector.tensor_scalar(
                            out=ot[:, h * D : (h + 1) * D],
                            in0=pt[:, h * D : (h + 1) * D],
                            scalar1=rcp[:, h : h + 1], scalar2=0.0,
                            op0=mybir.AluOpType.mult,
                            op1=mybir.AluOpType.max)
                    nc.sync.dma_start(
                        out=out[g * 128 : (g + 1) * 128, :], in_=ot[:])
    return out


# ---------------------------------------------------------------- entry point
def kernel(feat_src, feat_dst, w_src, w_dst, attn, src_idx, dst_idx,
           _n_cores=N_CORES, _backend="hw", _results_hook=None,
           _runner=None):
    feat_src = np.asarray(feat_src, np.float32)
    feat_dst = np.asarray(feat_dst, np.float32)
    w_src = np.asarray(w_src, np.float32)
    w_dst = np.asarray(w_dst, np.float32)
    attn = np.asarray(attn, np.float32)
    src_idx = np.asarray(src_idx).astype(np.int32)
    dst_idx = np.asarray(dst_idx).astype(np.int32)

    cfg, in_maps, order = _prep(feat_src, feat_dst, w_src, w_dst, attn,
                                src_idx, dst_idx, _n_cores)

    nc = bacc.Bacc("TRN2", target_bir_lowering=False, debug=False)
    with tile.TileContext(nc) as tc:
        _build(nc, tc, cfg)
    nc.compile()

    if _backend == "sim":
        from concourse.bass_interp import CoreSim
        results = []
        for c in range(_n_cores):
            sim = CoreSim(nc, trace=False)
            for name, arr in in_maps[c].items():
                sim.tensor(name)[:] = arr
            sim.simulate(check_with_hw=False)
            results.append({"out": np.array(sim.tensor("out"))})
        res_obj = None
    elif _runner is not None:
        results = _runner(nc, in_maps)
        res_obj = None
    else:
        res_obj = run_bass_kernel_spmd(nc, in_maps,
                                       core_ids=list(range(_n_cores)))
        results = res_obj.results
    if _results_hook is not None:
        _results_hook(res_obj)

    # unpermute slot-ordered outputs back to dst ids
    n_dst = cfg["n_dst"]
    ndc = cfg["ndc"]
    out_full = np.zeros((n_dst, HD), np.float32)
    for c in range(_n_cores):
        oc = results[c]["out"].reshape(cfg["nwin"], W_DST, HD)
        n_here = min(ndc, n_dst - c * ndc)
        for r in range(cfg["nwin"]):
            w = int(order[c][r])
            d0 = w * W_DST
            n = min(W_DST, n_here - d0)
            if n > 0:
                out_full[c * ndc + d0 : c * ndc + d0 + n] = oc[r, :n]
    return out_full
